# revision 15
# baseline (speedup 1.0000x reference)
"""GraphSAGE (2x SAGEConv mean-agg + linear/BN/tanh heads) on 8 TRN2 NeuronCores.

Strategy (hardcoded for N=50000 nodes, E=800000 edges, F=128):
  - dst-shard nodes across cores: core c owns node rows [c*6272, (c+1)*6272).
  - host: partition edges by owner core, per 128-node tile, sorted; split each
    tile's edge list by src < 32768 (int16 gather-index limit) into lo/hi
    groups; pad groups with dummy (src=0, w=0) edges to a static chunk count.
  - device per tile: bulk dma_gather of neighbor feature rows (one row per
    edge), build scaled one-hot S_w[e, n] = (iota[n]==dst_local[e]) * w[e] on
    the vector engine, and accumulate meanT[f, n] += X_g^T @ S_w on the PE.
    Then h = relu(mean @ w1l + x_own @ w1r + b1l).
  - AllGather h across the 8 cores (DRAM collective), layer 2 gathers from the
    gathered h, heads are computed from the transposed layer-2 output with
    BN / true_lab folded into the weights on the host.
"""

import math

import numpy as np

import concourse.bass as bass
import concourse.mybir as mybir
import concourse.tile as tile
from concourse.bass_utils import run_bass_kernel_spmd
from concourse.masks import make_identity
from concourse import library_config

F = 128
AF = mybir.ActivationFunctionType
ALU = mybir.AluOpType
f32 = mybir.dt.float32
i16 = mybir.dt.int16


class Cfg:
    def __init__(self, n_cores, tiles, ctot):
        self.n_cores = n_cores
        self.tiles = tiles          # 128-node tiles per core
        self.npc = tiles * 128      # nodes per core
        self.npad = n_cores * self.npc
        self.ctot = ctot            # 128-edge chunks per tile

    def key(self):
        return (self.n_cores, self.tiles, self.ctot)


def _build_program(cfg: Cfg):
    nc = bass.Bass(num_devices=cfg.n_cores, name="sage")

    # --- DRAM I/O ---
    feat_all = nc.dram_tensor("feat_all", [cfg.npad, F], f32, kind="ExternalInput")
    feat_own = nc.dram_tensor("feat_own", [cfg.npc, F], f32, kind="ExternalInput")
    idx32 = nc.dram_tensor("idx32", [cfg.tiles, 128, cfg.ctot], mybir.dt.int32, kind="ExternalInput")
    dlocw = nc.dram_tensor("dlocw", [cfg.tiles, 128, 2 * cfg.ctot], f32, kind="ExternalInput")
    w1l = nc.dram_tensor("w1l", [F, F], f32, kind="ExternalInput")
    w1r = nc.dram_tensor("w1r", [F, F], f32, kind="ExternalInput")
    b1l = nc.dram_tensor("b1l", [1, F], f32, kind="ExternalInput")
    w2l = nc.dram_tensor("w2l", [F, F], f32, kind="ExternalInput")
    w2r = nc.dram_tensor("w2r", [F, F], f32, kind="ExternalInput")
    b2l = nc.dram_tensor("b2l", [F, 1], f32, kind="ExternalInput")
    whead = nc.dram_tensor("whead", [F, 465], f32, kind="ExternalInput")
    bhead = nc.dram_tensor("bhead", [1, 465], f32, kind="ExternalInput")
    wcv = nc.dram_tensor("wcv", [64, 100], f32, kind="ExternalInput")
    bcv = nc.dram_tensor("bcv", [1, 100], f32, kind="ExternalInput")
    iota_in = nc.dram_tensor("iota", [128, 128], f32, kind="ExternalInput")

    o_fea_lab = nc.dram_tensor("o_fea_lab", [cfg.npc, 300], f32, kind="ExternalOutput")
    o_log = nc.dram_tensor("o_log", [cfg.npc, 100], f32, kind="ExternalOutput")
    o_out = nc.dram_tensor("o_out", [cfg.npc, 64], f32, kind="ExternalOutput")
    o_fc = nc.dram_tensor("o_fc", [cfg.npc, 100], f32, kind="ExternalOutput")
    o_true = nc.dram_tensor("o_true", [cfg.npc, 1], f32, kind="ExternalOutput")

    with tile.TileContext(nc) as tc:
        from contextlib import ExitStack

        with ExitStack() as ctx:
            cpool = ctx.enter_context(tc.tile_pool(name="consts", bufs=1))
            iopool = ctx.enter_context(tc.tile_pool(name="io", bufs=3))
            xgpool = ctx.enter_context(tc.tile_pool(name="xg", bufs=2))
            swpool = ctx.enter_context(tc.tile_pool(name="sw", bufs=4))
            eppool = ctx.enter_context(tc.tile_pool(name="ep", bufs=3))
            psA = ctx.enter_context(tc.tile_pool(name="psA", bufs=2, space="PSUM"))
            psB = ctx.enter_context(tc.tile_pool(name="psB", bufs=3, space="PSUM"))
            psC = ctx.enter_context(tc.tile_pool(name="psC", bufs=3, space="PSUM"))

            # constants
            w1l_s = cpool.tile_from(w1l[:])
            w1r_s = cpool.tile_from(w1r[:])
            b1l_s = cpool.tile_from(b1l[:])
            w2l_s = cpool.tile_from(w2l[:])
            w2r_s = cpool.tile_from(w2r[:])
            b2l_s = cpool.tile_from(b2l[:])
            whead_s = cpool.tile_from(whead[:])
            bhead_s = cpool.tile_from(bhead[:])
            wcv_s = cpool.tile_from(wcv[:])
            bcv_s = cpool.tile_from(bcv[:])
            iota_s = cpool.tile_from(iota_in[:])
            ident_s = cpool.tile([128, 128], f32)
            make_identity(nc, ident_s[:])
            ones1_s = cpool.tile([1, 128], f32)
            nc.vector.memset(ones1_s[:], 1.0)

            # one-time warmups: absorb constant-load DMA sems on engines whose
            # instructions have a single sync-wait slot (DVE) or few (ACT).
            warm_s = cpool.tile([128, 1], f32)
            nc.vector.tensor_copy(warm_s[:], iota_s[:, 0:1])
            warm2_s = cpool.tile([128, 1], f32)
            nc.scalar.copy(warm2_s[:], b2l_s[:, 0:1])

            # resident transposed h of the own slice: [hid, tiles*128]
            hT_res = cpool.tile([128, cfg.tiles * 128], f32)

            dpool = ctx.enter_context(tc.tile_pool(name="dram", bufs=1, space="DRAM"))
            hslice_d = dpool.tile([cfg.npc, F], f32, name="hslice_d")
            hfull_d = dpool.tile([cfg.npad, F], f32, name="hfull_d",
                                 addr_space="Shared" if cfg.n_cores > 4 else "Local")

            def agg_meanT(t, src_ap):
                """Gather + weighted one-hot matmul accumulation for tile t.
                Returns SBUF tile meanT [F, 128 nodes]."""
                ix_t = iopool.tile([128, cfg.ctot], mybir.dt.int32, tag="ix")
                nc.sync.dma_start(out=ix_t[:], in_=idx32[t, :, :])
                dlw_t = iopool.tile([128, 2 * cfg.ctot], f32, tag="dlw")
                nc.sync.dma_start(out=dlw_t[:], in_=dlocw[t, :, :])

                xg = xgpool.tile([128, cfg.ctot, 128], f32, tag="xg")
                for c in range(cfg.ctot):
                    nc.gpsimd.indirect_dma_start(
                        out=xg[:, c, :],
                        out_offset=None,
                        in_=src_ap,
                        in_offset=bass.IndirectOffsetOnAxis(ap=ix_t[:, c : c + 1], axis=0),
                    )

                # absorb the dlw DMA-completion wait on DVE: TensorScalarPtr has a
                # single sync-wait slot, so the chunk ops below must not need it.
                touch = iopool.tile([128, 1], f32, tag="touch")
                nc.vector.tensor_copy(touch[:], dlw_t[:, 0:1])
                ps_mean = psA.tile([128, 128], f32, tag="mean")
                for c in range(cfg.ctot):
                    sw = swpool.tile([128, 128], f32, tag="sw")
                    nc.vector.tensor_scalar(
                        sw[:],
                        iota_s[:],
                        dlw_t[:, c : c + 1],
                        dlw_t[:, cfg.ctot + c : cfg.ctot + c + 1],
                        op0=ALU.is_equal,
                        op1=ALU.mult,
                    )
                    nc.tensor.matmul(
                        ps_mean[:],
                        lhsT=xg[:, c, :],
                        rhs=sw[:],
                        start=(c == 0),
                        stop=(c == cfg.ctot - 1),
                    )
                meanT_s = eppool.tile([128, 128], f32, tag="meanT")
                nc.scalar.copy(meanT_s[:], ps_mean[:])
                return meanT_s

            # ---------------- layer 1 ----------------
            for t in range(cfg.tiles):
                meanT_s = agg_meanT(t, feat_all[:, :])

                xown_s = iopool.tile([128, F], f32, tag="xown")
                nc.sync.dma_start(out=xown_s[:], in_=feat_own[t * 128 : (t + 1) * 128, :])
                ps_t = psC.tile([128, 128], f32, tag="c")
                nc.tensor.transpose(ps_t[:], xown_s[:], ident_s[:])
                xownT_s = eppool.tile([128, 128], f32, tag="xownT")
                nc.scalar.copy(xownT_s[:], ps_t[:])

                ps_h = psB.tile([128, 128], f32, tag="b")
                nc.tensor.matmul(ps_h[:], lhsT=meanT_s[:], rhs=w1l_s[:], start=True, stop=False)
                nc.tensor.matmul(ps_h[:], lhsT=xownT_s[:], rhs=w1r_s[:], start=False, stop=False)
                nc.tensor.matmul(ps_h[:], lhsT=ones1_s[:], rhs=b1l_s[:], start=False, stop=True)
                h_s = eppool.tile([128, 128], f32, tag="h_s")
                nc.scalar.activation(h_s[:], ps_h[:], AF.Relu)
                nc.sync.dma_start(out=hslice_d[t * 128 : (t + 1) * 128, :], in_=h_s[:])

                ps_t2 = psC.tile([128, 128], f32, tag="c")
                nc.tensor.transpose(ps_t2[:], h_s[:], ident_s[:])
                nc.scalar.copy(hT_res[:, t * 128 : (t + 1) * 128], ps_t2[:])

            # ---------------- halo exchange ----------------
            nc.gpsimd.collective_compute(
                "AllGather",
                ALU.bypass,
                replica_groups=[list(range(cfg.n_cores))],
                ins=[hslice_d[:, :]],
                outs=[hfull_d[:, :]],
            )

            # ---------------- layer 2 + heads ----------------
            for t in range(cfg.tiles):
                meanT_s = agg_meanT(t, hfull_d[:, :])

                ps_h2 = psB.tile([128, 128], f32, tag="b")
                nc.tensor.matmul(ps_h2[:], lhsT=w2l_s[:], rhs=meanT_s[:], start=True, stop=False)
                nc.tensor.matmul(
                    ps_h2[:],
                    lhsT=w2r_s[:],
                    rhs=hT_res[:, t * 128 : (t + 1) * 128],
                    start=False,
                    stop=True,
                )
                h2T_s = eppool.tile([128, 128], f32, tag="h2T")
                nc.scalar.activation(h2T_s[:], ps_h2[:], AF.Relu, bias=b2l_s[:, 0:1])

                ps_hd = psB.tile([128, 465], f32, tag="b")
                nc.tensor.matmul(ps_hd[:], lhsT=h2T_s[:], rhs=whead_s[:], start=True, stop=False)
                nc.tensor.matmul(ps_hd[:], lhsT=ones1_s[:], rhs=bhead_s[:], start=False, stop=True)

                rows = slice(t * 128, (t + 1) * 128)
                fl_s = eppool.tile([128, 300], f32, tag="fl")
                nc.scalar.copy(fl_s[:], ps_hd[:, 0:300])
                nc.sync.dma_start(out=o_fea_lab[rows, :], in_=fl_s[:])
                lg_s = eppool.tile([128, 100], f32, tag="lg")
                nc.scalar.copy(lg_s[:], ps_hd[:, 300:400])
                nc.sync.dma_start(out=o_log[rows, :], in_=lg_s[:])
                tr_s = eppool.tile([128, 1], f32, tag="tl")
                nc.scalar.copy(tr_s[:], ps_hd[:, 464:465])
                nc.sync.dma_start(out=o_true[rows, :], in_=tr_s[:])

                ot_s = eppool.tile([128, 64], f32, tag="ot")
                nc.scalar.activation(ot_s[:], ps_hd[:, 400:464], AF.Tanh)
                nc.sync.dma_start(out=o_out[rows, :], in_=ot_s[:])

                ps_t3 = psC.tile([64, 128], f32, tag="c")
                nc.tensor.transpose(ps_t3[:], ot_s[:], ident_s[:])
                otT_s = eppool.tile([64, 128], f32, tag="otT")
                nc.scalar.copy(otT_s[:], ps_t3[:])

                ps_fc = psC.tile([128, 100], f32, tag="c")
                nc.tensor.matmul(ps_fc[:], lhsT=otT_s[:], rhs=wcv_s[:], start=True, stop=False)
                nc.tensor.matmul(ps_fc[:], lhsT=ones1_s[:], rhs=bcv_s[:], start=False, stop=True)
                fc_s = eppool.tile([128, 100], f32, tag="fc_s")
                nc.scalar.copy(fc_s[:], ps_fc[:])
                nc.sync.dma_start(out=o_fc[rows, :], in_=fc_s[:])

    _split_multi_waits(nc)
    return nc


def _split_multi_waits(nc):
    """This toolchain's codegen only supports ONE sync-wait per instruction.
    Split extra waits onto same-engine NoOps inserted right before."""
    n = 0
    for f in nc.m.functions:
        for bb in f.blocks:
            need = any(
                ins.sync_info is not None and ins.sync_info.on_wait and len(ins.sync_info.on_wait) > 1
                for ins in bb.instructions
            )
            if not need:
                continue
            out = []
            for ins in bb.instructions:
                si = ins.sync_info
                if si is not None and si.on_wait and len(si.on_wait) > 1:
                    waits = list(si.on_wait)
                    for w in waits[:-1]:
                        n += 1
                        nop = mybir.InstNoOp(
                            name=f"I-wsplit-{n}",
                            engine=ins.engine,
                            ins=[],
                            outs=[],
                            sync_info=mybir.SyncInfo(on_wait=[w], on_update=[]),
                        )
                        nc.register_instruction(nop, overwrite=True)
                        out.append(nop)
                    ins.sync_info = mybir.SyncInfo(on_wait=[waits[-1]], on_update=list(si.on_update or []))
                out.append(ins)
            bb.instructions = out
    return n


def _prep_host(features, edges, n_nodes, cfg_hint):
    """Edge partitioning + padding. Returns (cfg, per_core_arrays, shared_arrays)."""
    n_cores, tiles = cfg_hint
    npc = tiles * 128
    npad = n_cores * npc

    src = edges[0].astype(np.int64)
    dst = edges[1].astype(np.int64)
    deg = np.bincount(dst, minlength=n_nodes)
    w_edge = (1.0 / np.maximum(deg, 1.0))[dst].astype(np.float32)

    owner = dst // npc
    t_in = (dst % npc) // 128
    dloc = (dst % 128).astype(np.float32)

    key = owner * tiles + t_in
    order = np.argsort(key, kind="stable")
    counts = np.bincount(key, minlength=n_cores * tiles)
    offs = np.concatenate([[0], np.cumsum(counts)])
    ctot = max(1, math.ceil(counts.max() / 128))
    cfg = Cfg(n_cores, tiles, ctot)

    ssrc = src[order]
    sdloc = dloc[order]
    sw = w_edge[order]

    idx32 = np.zeros((n_cores, tiles, 128, ctot), np.int32)
    dlocw = np.zeros((n_cores, tiles, 128, 2 * ctot), np.float32)

    gi = 0
    for c in range(n_cores):
        for t in range(tiles):
            beg, end = offs[gi], offs[gi + 1]
            gi += 1
            n = end - beg
            if n == 0:
                continue
            i = np.arange(n)
            idx32[c, t, i % 128, i // 128] = ssrc[beg:end]
            dlocw[c, t, i % 128, i // 128] = sdloc[beg:end]
            dlocw[c, t, i % 128, ctot + i // 128] = sw[beg:end]

    feat_pad = np.zeros((npad, F), np.float32)
    feat_pad[:n_nodes] = features

    shared = {
        "feat_all": feat_pad,
        "iota": np.tile(np.arange(128, dtype=np.float32), (128, 1)),
    }
    per_core = []
    for c in range(n_cores):
        per_core.append(
            {
                "feat_own": feat_pad[c * npc : (c + 1) * npc].copy(),
                "idx32": idx32[c],
                "dlocw": dlocw[c],
            }
        )
    return cfg, per_core, shared


def _fold_weights(inp):
    eps = 1e-5
    s = (inp["gamma"] / np.sqrt(inp["rv"] + eps)).astype(np.float32)
    wbn = (inp["wconv"] * s[None, :]).astype(np.float32)
    bbn = ((inp["bconv"] - inp["rm"]) * s + inp["beta"]).astype(np.float32)
    wtl2 = (wbn @ inp["wtl"]).astype(np.float32)
    btl2 = (bbn @ inp["wtl"] + inp["btl"]).astype(np.float32)
    whead = np.concatenate([inp["whd"], inp["wclas"], wbn, wtl2], axis=1)
    bhead = np.concatenate([inp["bhd"], inp["bclas"], bbn, btl2])[None, :]
    return {
        "w1l": np.asarray(inp["w1l"], np.float32),
        "w1r": np.asarray(inp["w1r"], np.float32),
        "b1l": np.asarray(inp["b1l"], np.float32)[None, :],
        "w2l": np.asarray(inp["w2l"], np.float32),
        "w2r": np.asarray(inp["w2r"], np.float32),
        "b2l": np.asarray(inp["b2l"], np.float32)[:, None],
        "whead": np.ascontiguousarray(whead, np.float32),
        "bhead": np.ascontiguousarray(bhead, np.float32),
        "wcv": np.asarray(inp["wcv"], np.float32),
        "bcv": np.asarray(inp["bcv"], np.float32)[None, :],
    }


_prog_cache = {}


def run_sharded(inputs, n_nodes, cfg_hint, trace=False):
    features = np.asarray(inputs["features"], np.float32)
    edges = np.asarray(inputs["edges"])
    cfg, per_core, shared = _prep_host(features, edges, n_nodes, cfg_hint)
    folded = _fold_weights({k: np.asarray(v) for k, v in inputs.items() if k not in ("features", "edges")})

    k = cfg.key()
    if k not in _prog_cache:
        _prog_cache[k] = _build_program(cfg)
    nc = _prog_cache[k]

    in_maps = []
    for c in range(cfg.n_cores):
        m = {}
        m.update(shared)
        m.update(per_core[c])
        m.update(folded)
        in_maps.append(m)

    try:
        res = run_bass_kernel_spmd(nc, in_maps, core_ids=list(range(cfg.n_cores)), trace=trace)
    except ModuleNotFoundError:
        # axon NTFF profiling hook unavailable in this container
        res = run_bass_kernel_spmd(nc, in_maps, core_ids=list(range(cfg.n_cores)), trace=False)
    outs = res.results

    def cat(name):
        return np.concatenate([outs[c][name] for c in range(cfg.n_cores)], axis=0)[:n_nodes]

    logists = cat("o_log")
    out_t = cat("o_out")
    fea_lab = cat("o_fea_lab")
    fea_convert = cat("o_fc")
    true_lab = cat("o_true")
    return (logists, out_t, fea_lab, fea_convert, true_lab), res


def kernel(**inputs):
    result, _ = run_sharded(inputs, n_nodes=50000, cfg_hint=(8, 49))
    return result
